# revision 14
# baseline (speedup 1.0000x reference)
"""Trainium2 Bass kernel for nn_AverageAttention: cumulative-average attention
with a sigmoid gating Linear(2D->2D).

Strategy: data-parallel over batch (B=8 = one batch element per NeuronCore).
All on-chip work happens in transposed space ([feature, token]) because the
TensorEngine contracts over the partition dim:
  - cumavg via the affine recurrence avg_t = coef_t*avg_{t-1} + x_t/(t+1),
    one fused tensor_tensor_scan per 512-col chunk on VectorE (host
    pre-scales xdiv = x/(t+1)); chunks chained through a carry tile; the
    bf16 cast for the matmul stays on VectorE right behind each scan
  - gates^T[o,t] = sum_k W^T-tile @ G^T[k,t] with W stationary (host
    pre-tiles W into one contiguous 2MB block per output tile, bf16),
    G = concat(x, avg) resident in SBUF as bf16
  - unit order: all 16 output tiles at t-slice 0 first (W streamed twice
    total on a dedicated DMA queue) so slice 1-3 scans hide behind
    slice-0 matmuls; scan-sets for slices 1-3 are EMITTED between pass-1
    unit groups so no in-order engine stream blocks early epilogues;
    the first two units issue their x-half matmuls as a scan-free runway
  - DMA queues: W on sync HWDGE, x on scalar HWDGE, xd/avg/out on
    gpsimd SWDGE
  - sigmoid+bias fused on ScalarE reading PSUM, combine on VectorE,
    outputs written transposed and un-transposed on host.
"""
import sys

if "/opt/trn_rl_repo" not in sys.path:
    sys.path.insert(0, "/opt/trn_rl_repo")

import numpy as np
import ml_dtypes

B, T, D = 8, 2048, 2048
O = 2 * D          # gate output features (4096)
P = 128            # partitions
KT = D // P        # 16 k-tiles per half of G
DT = D // P        # 16 output-feature tiles
NK = 2 * KT        # 32 k-tiles total
TS = 512           # t-slice (matmul moving free dim / scan chunk)
NS = T // TS       # 4 t-slices
RUNWAY = 3         # units whose x-half matmuls front-run the scans

_compiled = None


def _build():
    import concourse.mybir as mybir
    import concourse.tile as tile
    from concourse import bacc

    f32 = mybir.dt.float32
    bf16 = mybir.dt.bfloat16
    SIG = mybir.ActivationFunctionType.Sigmoid

    nc = bacc.Bacc(trn_type="TRN2", target_bir_lowering=False, debug=False,
                   num_devices=B)

    xT_d = nc.declare_dram_parameter("xT", [D, T], bf16, isOutput=False)
    xdT_d = nc.declare_dram_parameter("xdT", [D, T], bf16, isOutput=False)
    wP_d = nc.declare_dram_parameter("wP", [DT, O, 2 * P], bf16,
                                     isOutput=False)
    bias_d = nc.declare_dram_parameter("bias", [O], f32, isOutput=False)
    coef_d = nc.declare_dram_parameter("coef_t", [1, T], f32, isOutput=False)
    avgT_d = nc.declare_dram_parameter("avgT", [D, T], bf16, isOutput=True)
    outT_d = nc.declare_dram_parameter("outT", [D, T], f32, isOutput=True)

    with tile.TileContext(nc) as tc:
        with tc.tile_pool(name="consts", bufs=1) as consts, \
             tc.tile_pool(name="resid", bufs=1) as resid, \
             tc.tile_pool(name="xdp", bufs=13) as xdp, \
             tc.tile_pool(name="wpool", bufs=3) as wpool, \
             tc.tile_pool(name="sigp", bufs=3) as sigp, \
             tc.tile_pool(name="outp", bufs=2) as outp, \
             tc.tile_pool(name="psum", bufs=8, space="PSUM") as pp:

            def load_w(i, split=1):
                w_i = wpool.tile([P, NK, 2 * P], bf16, tag="w")
                src_w = wP_d[i].rearrange("(kt p) c -> p kt c", p=P)
                step = NK // split
                for c in range(split):
                    ks = slice(c * step, (c + 1) * step)
                    nc.sync.dma_start(out=w_i[:, ks, :], in_=src_w[:, ks, :])
                return w_i

            # coef first on the scalar HWDGE queue — scans need it early
            coef_sb = consts.tile([P, T], f32)
            nc.scalar.dma_start(out=coef_sb,
                                in_=coef_d[:, :].to_broadcast((P, T)))

            # W for the runway units first — PE's earliest dependency;
            # w_0 in 4 chunks so the first matmul starts after 512KB
            w_tiles = {0: load_w(0, split=8)}
            for i in range(1, RUNWAY):
                w_tiles[i] = load_w(i)

            bias_sb = consts.tile([P, O // P], f32)
            nc.sync.dma_start(
                out=bias_sb, in_=bias_d.rearrange("(c p) -> p c", p=P))
            carry = consts.tile([P, KT], f32)

            xT_bf = resid.tile([P, KT, T], bf16)
            avgT_bf = resid.tile([P, KT, T], bf16)

            def scan_set(s):
                """Phase-A ops for slice s. All 16 xd loads are issued
                before the scan loop so the in-order gpsimd stream never
                blocks a later xd issue behind a scan-dependent avg
                write — scans then run back-to-back at DVE speed."""
                sl = slice(s * TS, (s + 1) * TS)
                xds = []
                for j in range(KT):
                    rows = slice(j * P, (j + 1) * P)
                    xd = xdp.tile([P, TS], bf16, tag="xd")
                    nc.gpsimd.dma_start(out=xd, in_=xdT_d[rows, sl])
                    nc.scalar.dma_start(out=xT_bf[:, j, sl],
                                        in_=xT_d[rows, sl])
                    xds.append(xd)
                for j in range(KT):
                    rows = slice(j * P, (j + 1) * P)
                    avc = avgT_bf[:, j, sl]
                    nc.vector.tensor_tensor_scan(
                        out=avc, data0=coef_sb[:, sl], data1=xds[j],
                        initial=(0.0 if s == 0 else carry[:, j:j + 1]),
                        op0=mybir.AluOpType.mult, op1=mybir.AluOpType.add)
                    if s < NS - 1:
                        nc.vector.tensor_copy(carry[:, j:j + 1],
                                              avc[:, TS - 1:TS])
                    nc.gpsimd.dma_start(out=avgT_d[rows, sl], in_=avc)

            def mm_half(ps_ig, ps_fg, w_i, s, half):
                sl = slice(s * TS, (s + 1) * TS)
                ks = range(0, KT) if half == 0 else range(KT, NK)
                for k in ks:
                    rhs = (xT_bf[:, k, sl] if k < KT
                           else avgT_bf[:, k - KT, sl])
                    nc.tensor.matmul(ps_ig, lhsT=w_i[:, k, 0:P], rhs=rhs,
                                     start=(k == 0), stop=(k == NK - 1))
                for k in ks:
                    rhs = (xT_bf[:, k, sl] if k < KT
                           else avgT_bf[:, k - KT, sl])
                    nc.tensor.matmul(ps_fg, lhsT=w_i[:, k, P:2 * P], rhs=rhs,
                                     start=(k == 0), stop=(k == NK - 1))

            def epilogue(ps_ig, ps_fg, i, s):
                sl = slice(s * TS, (s + 1) * TS)
                sig_i = sigp.tile([P, TS], f32, tag="sig")
                nc.scalar.activation(sig_i, ps_ig, SIG,
                                     bias=bias_sb[:, i:i + 1])
                sig_f = sigp.tile([P, TS], f32, tag="sig")
                nc.scalar.activation(sig_f, ps_fg, SIG,
                                     bias=bias_sb[:, KT + i:KT + i + 1])
                out_s = outp.tile([P, TS], f32, tag="out")
                nc.vector.tensor_mul(out_s, sig_i, xT_bf[:, i, sl])
                nc.vector.tensor_mul(sig_f, sig_f, avgT_bf[:, i, sl])
                nc.vector.tensor_add(out_s, out_s, sig_f)
                nc.scalar.dma_start(out=outT_d[i * P:(i + 1) * P, sl],
                                     in_=out_s)

            def full_unit(w_i, i, s):
                ps_ig = pp.tile([P, TS], f32, tag="ps")
                ps_fg = pp.tile([P, TS], f32, tag="ps")
                mm_half(ps_ig, ps_fg, w_i, s, half=0)
                mm_half(ps_ig, ps_fg, w_i, s, half=1)
                epilogue(ps_ig, ps_fg, i, s)

            # ---- pass 1 (s = 0 across all i), interleaved with the
            # ---- remaining scan-sets so no engine stream head-blocks
            scan_set(0)
            run_ps = []
            for i in range(RUNWAY):
                ps_ig = pp.tile([P, TS], f32, tag="ps")
                ps_fg = pp.tile([P, TS], f32, tag="ps")
                mm_half(ps_ig, ps_fg, w_tiles[i], 0, half=0)
                run_ps.append((ps_ig, ps_fg))
            for i in range(RUNWAY):
                ps_ig, ps_fg = run_ps[i]
                mm_half(ps_ig, ps_fg, w_tiles[i], 0, half=1)
                epilogue(ps_ig, ps_fg, i, 0)
            scan_set(1)
            for i in range(RUNWAY, 6):
                full_unit(load_w(i), i, 0)
            scan_set(2)
            for i in range(6, 10):
                full_unit(load_w(i), i, 0)
            scan_set(3)
            p1_tiles = {}
            for i in range(10, DT):
                p1_tiles[i] = load_w(i)
                full_unit(p1_tiles[i], i, 0)

            # ---- pass 2: remaining slices, descending i so the last
            # ---- three pass-1 W tiles (still resident) are reused ----
            for i in reversed(range(DT)):
                w_i = p1_tiles[i] if i >= DT - 3 else load_w(i)
                for s in range(1, NS):
                    full_unit(w_i, i, s)

    nc.compile()
    return nc


def _get_compiled():
    global _compiled
    if _compiled is None:
        _compiled = _build()
    return _compiled


def _run(inputs, trace=False, **spmd_kwargs):
    from concourse.bass_utils import run_bass_kernel_spmd

    nc = _get_compiled()
    layer_in = np.asarray(inputs["layer_in"], dtype=np.float32)
    W_gate = np.asarray(inputs["W_gate"], dtype=np.float32)
    b_gate = np.asarray(inputs["b_gate"], dtype=np.float32)

    # wP[i, k, 0:128] = W^T[k, i*128:(i+1)*128]     (input-gate columns)
    # wP[i, k, 128:256] = W^T[k, D+i*128:D+(i+1)*128]  (forget-gate columns)
    wT = np.ascontiguousarray(W_gate.T)                    # [k, o]
    wP = np.ascontiguousarray(
        wT.reshape(O, 2, DT, P).transpose(2, 0, 1, 3).reshape(DT, O, 2 * P)
    ).astype(ml_dtypes.bfloat16)
    tt = np.arange(T, dtype=np.float32)
    coef = (tt / (tt + 1.0)).reshape(1, T)
    inv = (1.0 / (tt + 1.0)).reshape(1, T)

    in_maps = []
    for b in range(B):
        xTb = np.ascontiguousarray(layer_in[b].T)
        in_maps.append({
            "xT": xTb.astype(ml_dtypes.bfloat16),
            "xdT": (xTb * inv).astype(ml_dtypes.bfloat16),
            "wP": wP,
            "bias": b_gate,
            "coef_t": coef,
        })

    res = run_bass_kernel_spmd(nc, in_maps, core_ids=list(range(B)),
                               trace=trace, **spmd_kwargs)
    gating = np.empty((B, T, D), dtype=np.float32)
    avg = np.empty((B, T, D), dtype=np.float32)
    for b in range(B):
        gating[b] = res.results[b]["outT"].T
        avg[b] = res.results[b]["avgT"].astype(np.float32).T
    return (gating, avg), res


def kernel(**inputs):
    (gating, avg), _ = _run(inputs, trace=False)
    return gating, avg
